# revision 44
# baseline (speedup 1.0000x reference)
"""DenseCRF loss kernel for Trainium2 (8 NeuronCores, data-parallel over batch).

Per core (one image):
  downsample inputs (nearest for img/ROI, 2x2-avg for logS/log1_S),
  bilateral features f = [rgb/15, xy/50] (P=4096 pixels, 5 dims),
  W[q,p] = exp(f_q.f_p - 0.5|f_q|^2 - 0.5|f_p|^2)  (dense 4096x4096, tiled),
  loss_i = sum_k a_k . (W @ b_k),  a = logS_ds*roi, b = log1S_ds*roi.
Host sums the 8 per-image scalars and applies WEIGHT/N (and the a-side 0.25
bilinear-normalization factor; the b-side 0.25 is folded into b^T on-chip).

Math layout (all matmuls run at 1 cycle/row via float32r):
  G[q,p] = f_q.f_p is computed with rgb split hi/lo (hi = bf16-rounded, exactly
  representable in tf32, so hi*hi is exact; cross terms carry ~2^-20 error):
    lhsT rows:  @0 rgb_hi | @32 rgb_lo | @64 rgb_hi | @96 xy
    rhs  rows:  @0 rgb_hi | @32 rgb_hi | @64 rgb_lo | @96 xy
  (row groups start at partitions 0/32/64/96 - engine writes must start there)
  W' = Exp(G + bias_q) with bias_q = -0.5|f_q|^2 - M  (fp32, exact).
  The -0.5|f_p|^2 term is NOT in the matmul: it is folded into the a-mask as
  exp(M - 0.5|f_p|^2) (a column-p scale of W commutes with the q-contraction).
  M = 60 keeps exponents in fp32 range (centered rgb: 0.5*max|f|^2 ~ 110).
  PE:  acc[21,512] += bT_j^T @ W'  (b^T stationary: only 21 columns to load)
  DVE: flush = reduce(acc * amask) per 512-chunk of p.
"""
import os
import sys

import numpy as np

for _p in ("/opt/trn_rl_repo", "/root/.axon_site/_ro/trn_rl_repo"):
    if os.path.isdir(_p) and _p not in sys.path:
        sys.path.insert(0, _p)

import concourse.bass as bass
import concourse.bacc as bacc
import concourse.tile as tile
from concourse import mybir
from concourse.bass_utils import run_bass_kernel_spmd

F32 = mybir.dt.float32
F32R = mybir.dt.float32r
BF16 = mybir.dt.bfloat16
AF = mybir.ActivationFunctionType
ALU = mybir.AluOpType

N_CORES = 8
K = 21
H = 128           # input image size
HD = 64           # downsampled size
P = HD * HD       # 4096 pixels
NJ = 32           # q blocks of 128
SIGMA_RGB = 15.0
SXY = 50.0        # SIGMA_XY * SCALE
WEIGHT = 1e-9
MSHIFT = 60.0     # exponent shift: W' = exp(.. - M), amask *= exp(M - sq/2)
CD = 102          # contraction dim of the generator matmul (rows 0..101)


def _build_body(tc, img, lgs, l1s, roi, posc, possq, eye, out):
    nc = tc.nc
    bounce = nc.dram_tensor("bounce", [1, P], F32, kind="Internal")
    with (
        tc.tile_pool(name="singles", bufs=1) as singles,
        tc.tile_pool(name="stream", bufs=2) as stream,
        tc.tile_pool(name="wpool", bufs=6) as wpool,
        tc.tile_pool(name="scr", bufs=2) as scr,
    ):
        # ---------------- persistent sbuf tiles ----------------
        fL = singles.tile([128, P], BF16)   # stationary feature stack
        fR = singles.tile([128, P], BF16)   # moving feature stack
        biasP = singles.tile([128, NJ], F32)  # -0.5*sq - M in [qp, jb] layout
        bT = singles.tile([128, NJ * K], F32R)  # b^T blocks, cols j*21..j*21+20
        repae = singles.tile([K, P], F32)   # roi*exp(M-sq/2) bcast to 21 parts
        # bmask then amask share one slot (bmask dead once bT is built)
        bmask = singles.tile([K, P], F32, tag="mask", name="bmask")
        roirow = singles.tile([1, P], F32)  # roi_ds, later roi*exp(M-sq/2)
        rT = singles.tile([128, NJ], F32)   # roi_ds in [qp, jb] layout
        neg3 = singles.tile([3, 1], F32)
        neg2 = singles.tile([2, 1], F32)
        eyes = singles.tile([K, K], F32)
        lossb = singles.tile([K, 8], F32)
        ones1 = singles.tile([1, K], F32)
        mneg = singles.tile([128, 1], F32)   # -MSHIFT per partition
        mpos = singles.tile([1, 1], F32)     # +MSHIFT

        # ---------------- input DMAs + constants ----------------
        # (DMA APs must end in a contiguous dim -> stage even rows only,
        #  then stride the columns on-chip with engine APs)
        nc.gpsimd.memset(fL, 0.0)
        nc.gpsimd.memset(fR, 0.0)
        nc.vector.memset(neg3, -0.5)
        nc.vector.memset(neg2, -0.5)
        nc.vector.memset(lossb, 0.0)
        nc.vector.memset(ones1, 1.0)
        nc.vector.memset(mneg, -MSHIFT)
        nc.vector.memset(mpos, MSHIFT)
        nc.sync.dma_start(out=eyes, in_=eye[:, :])
        # xy/50 features, hi/lo split -> rows 96-101 of both stacks
        nc.sync.dma_start(out=fL[96:102, :], in_=posc[0][:, :])
        nc.sync.dma_start(out=fR[96:102, :], in_=posc[1][:, :])
        # roi_ds in [qp, jb] layout (4-byte-descriptor DMAs, small, one-time)
        nc.sync.dma_start(
            out=rT[0:64, :],
            in_=bass.AP(tensor=roi, offset=0, ap=[[2, HD], [4 * H, NJ]]))
        nc.sync.dma_start(
            out=rT[64:128, :],
            in_=bass.AP(tensor=roi, offset=2 * H, ap=[[2, HD], [4 * H, NJ]]))
        # rgb (centered below) and roi rows, from even-row staging halves
        rgbf = stream.tile([3, P], F32, tag="rawq", bufs=3, name="rgbf")
        for hh in range(2):
            imgh = stream.tile([3, P], F32, tag="imgh", bufs=1, name="imgh")
            src = bass.AP(tensor=img, offset=hh * HD * H,
                          ap=[[H * H, 3], [2 * H, 32], [1, H]])
            nc.sync.dma_start(out=imgh, in_=src)
            iv = imgh.rearrange("c (h w b) -> c h w b", w=HD, b=2)[:, :, :, 0]
            nc.scalar.mul(
                rgbf[:, hh * 2048:(hh + 1) * 2048].rearrange(
                    "c (h w) -> c h w", w=HD),
                iv, 1.0 / SIGMA_RGB)
            roih = stream.tile([1, P], F32, tag="roih", bufs=1, name="roih")
            rsrc = bass.AP(tensor=roi, offset=hh * HD * H,
                           ap=[[H * H, 1], [2 * H, 32], [1, H]])
            nc.sync.dma_start(out=roih, in_=rsrc)
            rv = roih.rearrange("c (h w b) -> c h w b", w=HD, b=2)[:, :, :, 0]
            nc.scalar.copy(
                roirow[:, hh * 2048:(hh + 1) * 2048].rearrange(
                    "c (h w) -> c h w", w=HD), rv)

        # quarters of the original [21,128,128] images: 32 orig rows each
        l1sq = l1s.rearrange("k (q r) w -> k q (r w)", q=4)
        lgsq = lgs.rearrange("k (q r) w -> k q (r w)", q=4)

        def quarter_sum(dst, src_q):
            """dst[21,1024] (16x64 px) = 4-view sum of src quarter [21,4096]."""
            v = src_q.rearrange("k (h a w b) -> k h a w b", a=2, b=2, w=HD)
            d = dst.rearrange("k (h w) -> k h w", w=HD)
            nc.vector.tensor_add(d, v[:, :, 0, :, 0], v[:, :, 0, :, 1])
            nc.vector.tensor_add(d, d, v[:, :, 1, :, 0])
            nc.vector.tensor_add(d, d, v[:, :, 1, :, 1])

        with tc.tile_pool(name="psetup", bufs=2, space="PSUM") as psetup:
            # ---------------- feature chain ----------------
            # center rgb per channel (keeps |f| and sq small: no fp32 exp
            # overflow with M=60, and shrinks the hi/lo split residuals)
            # (dummy copy target: bmask rows, overwritten later by b chain)
            rmean = scr.tile([3, 1], F32, tag="rmean", bufs=1, name="rmean")
            nc.scalar.activation(bmask[0:3, :], rgbf, AF.Copy,
                                 accum_out=rmean)
            nc.scalar.mul(rmean, rmean, 1.0 / P)
            nc.vector.tensor_scalar_sub(rgbf, rgbf, rmean)
            # hi = bf16-rounded rgb (exactly representable in tf32)
            rgbh = stream.tile([3, P], BF16, tag="rgbh", bufs=1, name="rgbh")
            nc.vector.tensor_copy(rgbh, rgbf)
            # place hi rows: fL@0, fL@64, fR@0, fR@32
            nc.scalar.copy(fL[0:3, :], rgbh)
            nc.scalar.copy(fR[0:3, :], rgbh)
            nc.vector.tensor_copy(fL[64:67, :], rgbh)
            nc.vector.tensor_copy(fR[32:35, :], rgbh)
            # lo rows: fL@32, fR@64  (lo = f - hi)
            nc.vector.tensor_sub(fL[32:35, :], rgbf, rgbh)
            nc.vector.tensor_sub(fR[64:67, :], rgbf, rgbh)

            # ---------------- sq chain ----------------
            rsq = stream.tile([3, P], F32, tag="rawq", bufs=3, name="rsq")
            psq = stream.tile([2, P], F32, tag="rawq", bufs=3, name="psq")
            nc.scalar.square(rsq, rgbf)
            nc.sync.dma_start(out=psq, in_=possq[:, :])
            # bias in [128, 32] layout: one [128,1] column per q-block
            setup_mms = []
            biasq_ps = psetup.tile([128, NJ], F32, bufs=1)
            for jb in range(NJ):
                jsl = slice(jb * 128, (jb + 1) * 128)
                setup_mms.append(nc.tensor.matmul(
                    biasq_ps[:, jb:jb + 1], lhsT=rsq[:, jsl],
                    rhs=neg3, start=True, stop=False))
                setup_mms.append(nc.tensor.matmul(
                    biasq_ps[:, jb:jb + 1], lhsT=psq[:, jsl],
                    rhs=neg2, start=False, stop=True))
            nc.scalar.activation(biasP, biasq_ps, AF.Identity, bias=mneg)
            # roirow *= exp(M - 0.5*sq) per p (exact fold of the -sq_p/2 term)
            ef = stream.tile([1, P], F32, tag="imgh", bufs=1, name="ef")
            for ch in range(8):
                csl = slice(ch * 512, (ch + 1) * 512)
                sq_ps = psetup.tile([1, 512], F32, tag="sqrow", name="sq_ps")
                setup_mms.append(nc.tensor.matmul(
                    sq_ps, lhsT=neg3, rhs=rsq[:, csl],
                    start=True, stop=False))
                setup_mms.append(nc.tensor.matmul(
                    sq_ps, lhsT=neg2, rhs=psq[:, csl],
                    start=False, stop=True))
                nc.scalar.activation(ef[:, csl], sq_ps, AF.Exp, bias=mpos)
            nc.vector.tensor_mul(roirow, roirow, ef)
            # broadcast to 21 partitions through a DRAM bounce (step-0
            # partition source is only legal from DRAM)
            nc.sync.dma_start(out=bounce[:, :], in_=roirow)
            nc.sync.dma_start(
                out=repae,
                in_=bass.AP(tensor=bounce, offset=0, ap=[[0, K], [1, P]]))

            # ------------- b chain: downsample (transposes are emitted
            # just-in-time inside the main loop to keep PE's in-order queue
            # free for generator matmuls) -------------
            for qq in range(4):
                rq = stream.tile([K, 4096], F32, tag="rawq", bufs=3, name="rq")
                nc.sync.dma_start(out=rq, in_=l1sq[:, qq, :])
                quarter_sum(bmask[:, qq * 1024:(qq + 1) * 1024], rq)

        # --------------- a chain (overlaps main loop on DVE) ---------------
        amask = singles.tile([K, P], F32, tag="mask", name="amask")
        for qq in range(4):
            rq = stream.tile([K, 4096], F32, tag="rawq", bufs=3, name="rq")
            nc.sync.dma_start(out=rq, in_=lgsq[:, qq, :])
            quarter_sum(amask[:, qq * 1024:(qq + 1) * 1024], rq)
        nc.vector.tensor_mul(amask, amask, repae)

        # ---------------- main loop ----------------
        nj_limit = int(os.environ.get("DCRF_NJ", NJ))
        spin = singles.tile([128, 512], BF16)
        nc.vector.memset(spin, 0.0)
        main_psum = (
            tc.tile_pool(name="pgen", bufs=2, space="PSUM"),
            tc.tile_pool(name="pacc", bufs=1, space="PSUM"),
        )
        pgen = main_psum[0].__enter__()
        pacc = main_psum[1].__enter__()

        def build_bt(jb):
            bt_ps = pgen.tile([128, K], F32, tag="gen", name="bt_ps")
            nc.tensor.matmul(
                bt_ps, lhsT=bmask[:, jb * 128:(jb + 1) * 128], rhs=eyes,
                start=True, stop=True,
            )
            # out = (psum * roi_q) * 0.25  (0.25 = bilinear-downsample
            # normalization for the b side; roi is per-q = per-partition)
            nc.vector.tensor_scalar(
                out=bT[:, jb * K:(jb + 1) * K], in0=bt_ps,
                scalar1=rT[:, jb:jb + 1], scalar2=0.25,
                op0=ALU.mult, op1=ALU.mult)

        accs = [pacc.tile([K, 512], F32, tag=f"acc{i}", name=f"acc{i}")
                for i in range(4)]
        for jb in range(4):
            build_bt(jb)
        import bass_rust as _br
        for si in range(12):
            sp_ps = pgen.tile([128, 512], F32, tag="gen", name="sp_ps")
            sp_mm = nc.tensor.matmul(sp_ps, lhsT=spin[:, 0:128], rhs=spin,
                                     start=True, stop=True)
            if si == 0:
                # order the PE queue: all setup matmuls (bias/sq chains)
                # before the warm-up spin and main loop, so the Exps don't
                # stall mid-kernel waiting for their bias columns
                for m in setup_mms:
                    _br.add_dep_helper(sp_mm.ins, m.ins, sync=False,
                                       reason="setup-before-main")
        for h in range(2):
            for j in range(nj_limit):
                jsl = slice(j * 128, (j + 1) * 128)
                bT_j = bT[:, j * K:(j + 1) * K]
                if h == 0 and j + 4 < nj_limit:
                    build_bt(j + 4)
                wts = []
                for t2 in range(2):
                    base = h * 2048 + t2 * 1024
                    gt = pgen.tile([128, 1024], F32, tag="gen")
                    for s in range(2):
                        nc.tensor.matmul(
                            gt[:, s * 512:(s + 1) * 512],
                            lhsT=fL[0:CD, jsl],
                            rhs=fR[0:CD, base + s * 512:base + (s + 1) * 512],
                            start=True, stop=True,
                        )
                    wt = wpool.tile([128, 1024], F32R, tag="wt")
                    nc.scalar.activation(wt, gt, AF.Exp,
                                         bias=biasP[:, j:j + 1], scale=1.0)
                    wts.append(wt)
                for t2, wt in enumerate(wts):
                    for s in range(2):
                        nc.tensor.matmul(
                            accs[t2 * 2 + s],
                            lhsT=bT_j,
                            rhs=wt[:, s * 512:(s + 1) * 512],
                            start=(j == 0), stop=(j == nj_limit - 1),
                        )
            # flush the 4 accumulators of this half
            for idx in range(4):
                li = h * 4 + idx
                c0 = h * 2048 + idx * 512
                ttr_out = scr.tile([K, 512], F32, tag="ttrout")
                nc.vector.tensor_mul(ttr_out, accs[idx], amask[:, c0:c0 + 512])
                nc.vector.reduce_sum(lossb[:, li:li + 1], ttr_out,
                                     axis=mybir.AxisListType.X)
        nc.sync.dma_start(out=out[:, :], in_=lossb[:, :])
        main_psum[1].__exit__(None, None, None)
        main_psum[0].__exit__(None, None, None)


def build_program():
    # Bacc (not plain Bass): its compile() pass splits multi-sem waits into
    # event-semaphore chains (TRN2 allows at most 1 wait per instruction).
    nc = bacc.Bacc("TRN2")
    img = nc.dram_tensor("img", [3, H, H], F32, kind="ExternalInput")
    lgs = nc.dram_tensor("lgs", [K, H, H], F32, kind="ExternalInput")
    l1s = nc.dram_tensor("l1s", [K, H, H], F32, kind="ExternalInput")
    roi = nc.dram_tensor("roi", [H, H], F32, kind="ExternalInput")
    poscl = nc.dram_tensor("poscl", [6, P], BF16, kind="ExternalInput")
    poscr = nc.dram_tensor("poscr", [6, P], BF16, kind="ExternalInput")
    possq = nc.dram_tensor("possq", [2, P], F32, kind="ExternalInput")
    eye = nc.dram_tensor("eye21", [K, K], F32, kind="ExternalInput")
    out = nc.dram_tensor("out", [K, 8], F32, kind="ExternalOutput")
    with tile.TileContext(nc) as tc:
        _build_body(tc, img, lgs, l1s, roi, (poscl, poscr), possq, eye, out)
    nc.finalize()
    return nc


_CACHE = {}


def _get_program():
    if "nc" not in _CACHE:
        _CACHE["nc"] = build_program()
    return _CACHE["nc"]


def _host_consts():
    import ml_dtypes
    bf16 = ml_dtypes.bfloat16
    yy, xx = np.meshgrid(np.arange(HD, dtype=np.float32),
                         np.arange(HD, dtype=np.float32), indexing="ij")
    pos = (np.stack([xx, yy], axis=0).reshape(2, P) / SXY).astype(np.float32)
    hi = pos.astype(bf16)
    lo = (pos - hi.astype(np.float32)).astype(bf16)
    poscl = np.concatenate([hi, lo, hi], axis=0)   # xhi yhi xlo ylo xhi yhi
    poscr = np.concatenate([hi, hi, lo], axis=0)   # xhi yhi xhi yhi xlo ylo
    possq = (pos.astype(np.float64) ** 2).astype(np.float32)
    return ((np.ascontiguousarray(poscl), np.ascontiguousarray(poscr)),
            np.ascontiguousarray(possq), np.eye(K, dtype=np.float32))


def run(images, logS, log1_S, ROIs, trace=False):
    nc = _get_program()
    posc, possq, eye = _host_consts()
    in_maps = []
    for i in range(N_CORES):
        in_maps.append({
            "img": np.ascontiguousarray(images[i], dtype=np.float32),
            "lgs": np.ascontiguousarray(logS[i], dtype=np.float32),
            "l1s": np.ascontiguousarray(log1_S[i], dtype=np.float32),
            "roi": np.ascontiguousarray(ROIs[i], dtype=np.float32),
            "poscl": posc[0],
            "poscr": posc[1],
            "possq": possq,
            "eye21": eye,
        })
    res = run_bass_kernel_spmd(nc, in_maps, core_ids=list(range(N_CORES)),
                               trace=trace)
    total = sum(float(r["out"].astype(np.float64).sum()) for r in res.results)
    # 0.25: bilinear-downsample normalization of the a side (b side on-chip)
    val = np.float32(WEIGHT * 0.25 * total / N_CORES)
    return np.asarray(val), res


def kernel(images, logS, log1_S, ROIs):
    return run(images, logS, log1_S, ROIs)[0]
